# revision 20
# baseline (speedup 1.0000x reference)
"""Channel-attention block (QKV 1x1 -> L2-normalized channel attention ->
depthwise 3x3 -> 1x1 proj) on 8 Trainium2 NeuronCores, data-parallel over
the batch (1 image per core).

Exact algebraic refactoring of the reference, per image x [C, N]:
    X   = x @ x.T                          (Gram; replaces the whole QKV)
    Aq  = Wq @ X ; nq2 = rowdot(Aq, Wq)    = diag(Wq X Wq^T) = |q_row|^2
    Ak  = Wk @ X ; nk2 = rowdot(Ak, Wk)
    rq  = temp / max(sqrt(nq2), eps) ; rk = 1 / max(sqrt(nk2), eps)
    G   = (diag(rq) Wq) X (diag(rk) Wk)^T  = attention logits
    attn = row-softmax of per-head 12x12 diagonal blocks of G
    v   = Wv @ x
    t   = blockdiag(attn) @ v
    z   = depthwise3x3(t);  y = Wp @ z

The depthwise 3x3 (per-channel scale x shifted view, 9 taps) is split
across engines: 2 taps folded into the projection matmul on PE (lhsT
pre-scaled by the tap weight, shifted rhs view, PSUM accumulates), 2
fused mul+adds on GpSimd, 2 scaled copies on ScalarE (merged by DVE),
3 on VectorE.

Channel layout: C=192 > 128 partitions, so channel-dim tensors are chunk
pairs [128,*]+[64,*] (or [96,*]x2 where head alignment matters). The
padded t for the depthwise conv is stored as three "stripes" of
[128, (H/2+2)*(W+2)] bf16: channels 0-127 top half, channels 0-127
bottom half, and channels 128-191 dual-packed (lanes 0-63 top half,
lanes 64-127 bottom half) so elementwise engines never run half-empty.

Self-contained: full unsharded inputs in, full output out.
"""

import numpy as np
from contextlib import ExitStack

B, C, H, W = 8, 192, 128, 128
HEADS = 16
HD = C // HEADS  # 12
EPS = 1e-12

ALL_TAPS = [(di, dj) for di in range(3) for dj in range(3)]
PE_TAPS = [(0, 1), (2, 1)]
# ACT computes scaled copies; merge engine per ACT tap listed alongside.
ACT_TAPS = [(1, 1), (2, 0), (0, 0)]
ACT_MERGE = ["pool", "dve", "dve"]
DVE_TAPS = [(1, 0), (0, 2), (1, 2), (2, 2)]  # first is the z-init mul
_NON_PE = ACT_TAPS + DVE_TAPS


def _bf16_np():
    import ml_dtypes
    return ml_dtypes.bfloat16


def make_plan(H_, W_):
    N = H_ * W_
    halfH = H_ // 2
    TPW = W_ + 2
    PR = max(1, min(512 // W_, halfH))
    while halfH % PR:
        PR -= 1
    RT = 2 * PR if halfH % (2 * PR) == 0 else PR
    NCH = N // 128
    assert N % 128 == 0
    XG = max(1, NCH // 8)
    while NCH % XG:
        XG -= 1
    return dict(H=H_, W=W_, N=N, halfH=halfH, TPW=TPW, PR=PR, RT=RT,
                NCH=NCH, XG=XG, SROWS=halfH + 2)


def host_prep(w_qkv, w_dw, w_proj, temperature):
    bf16 = _bf16_np()
    w_qkv = np.asarray(w_qkv, np.float32)
    wdw = np.asarray(w_dw, np.float32).reshape(C, 3, 3)
    w_proj = np.asarray(w_proj, np.float32)
    temp = np.asarray(temperature, np.float32).reshape(HEADS)

    Wq, Wk, Wv = w_qkv[:C], w_qkv[C:2 * C], w_qkv[2 * C:]
    ins = {
        "wqT": np.ascontiguousarray(Wq.T).astype(bf16),
        "wkT": np.ascontiguousarray(Wk.T).astype(bf16),
        "wvT": np.ascontiguousarray(Wv.T).astype(bf16),
        "wq_nat": Wq.astype(bf16),
        "wk_nat": Wk.astype(bf16),
        "temp_pc": np.repeat(temp, HD).reshape(C, 1).astype(np.float32),
        "ident": np.eye(128, dtype=np.float32).astype(bf16),
        "wpT": np.ascontiguousarray(w_proj.T).astype(bf16),
        "bmask": np.where(np.arange(C)[:, None] // HD == np.arange(C)[None] // HD,
                          0.0, -3e4).astype(np.float32),
    }
    for i, (di, dj) in enumerate(PE_TAPS):
        ins[f"wps{i}T"] = np.ascontiguousarray(
            w_proj.T * wdw[:, di, dj][:, None]).astype(bf16)
    ntap = len(_NON_PE)
    dwA = np.zeros((128, ntap), np.float32)
    dwB = np.zeros((128, ntap), np.float32)
    for k, (di, dj) in enumerate(_NON_PE):
        dwA[:, k] = wdw[:128, di, dj]
        dwB[:64, k] = wdw[128:, di, dj]
        dwB[64:, k] = wdw[128:, di, dj]
    ins["dwA"] = dwA
    ins["dwB"] = dwB
    return ins


def numpy_fold(x_img, w_qkv, w_dw, w_proj, temperature, plan):
    """Golden fp32 numpy model of the folded algorithm."""
    H_, W_, N = plan["H"], plan["W"], plan["N"]
    w_qkv = np.asarray(w_qkv, np.float32)
    wdw = np.asarray(w_dw, np.float32).reshape(C, 3, 3)
    Wp = np.asarray(w_proj, np.float32)
    temp = np.asarray(temperature, np.float32).reshape(HEADS)
    Wq, Wk, Wv = w_qkv[:C], w_qkv[C:2 * C], w_qkv[2 * C:]
    xf = np.asarray(x_img, np.float32).reshape(C, N)
    X = xf @ xf.T
    nq2 = ((Wq @ X) * Wq).sum(1)
    nk2 = ((Wk @ X) * Wk).sum(1)
    rq = np.repeat(temp, HD) / np.maximum(np.sqrt(np.maximum(nq2, EPS * EPS)), EPS)
    rk = 1.0 / np.maximum(np.sqrt(np.maximum(nk2, EPS * EPS)), EPS)
    G = (rq[:, None] * Wq) @ X @ (rk[:, None] * Wk).T
    A_bd = np.zeros((C, C), np.float32)
    for h in range(HEADS):
        s = slice(h * HD, (h + 1) * HD)
        g = G[s, s]
        g = g - g.max(1, keepdims=True)
        e = np.exp(g)
        A_bd[s, s] = e / e.sum(1, keepdims=True)
    t = (A_bd @ (Wv @ xf)).reshape(C, H_, W_)
    tp = np.pad(t, ((0, 0), (1, 1), (1, 1)))
    z = np.zeros_like(t)
    for di in range(3):
        for dj in range(3):
            z += wdw[:, di, dj][:, None, None] * tp[:, di:di + H_, dj:dj + W_]
    return (Wp @ z.reshape(C, N)).reshape(C, H_, W_)


# --------------------------------------------------------------------------
# device graph
# --------------------------------------------------------------------------

def build_graph(nc, tc, plan):
    import concourse.mybir as mybir
    from concourse.alu_op_type import AluOpType
    dt = mybir.dt
    AF = mybir.ActivationFunctionType
    AX = mybir.AxisListType
    f32, bf16 = dt.float32, dt.bfloat16

    H_, W_, N = plan["H"], plan["W"], plan["N"]
    halfH, TPW, PR, RT = plan["halfH"], plan["TPW"], plan["PR"], plan["RT"]
    NCH, XG, SROWS = plan["NCH"], plan["XG"], plan["SROWS"]
    PT = PR * W_
    NJT = N // PT
    HJT = NJT // 2
    NDT = halfH // RT
    RPT = RT // PR
    NGX = NCH // XG
    GPX = XG * 128          # pixels per load group
    SFREE = SROWS * TPW

    # ---- DRAM ----
    dram = {}
    def din(name, shape, dty):
        dram[name] = nc.dram_tensor(name, shape, dty, kind="ExternalInput").ap()
    din("x", [C, N], f32)
    for nm in ["wqT", "wkT", "wvT", "wq_nat", "wk_nat", "wpT"]:
        din(nm, [C, C], bf16)
    for i in range(len(PE_TAPS)):
        din(f"wps{i}T", [C, C], bf16)
    din("temp_pc", [C, 1], f32)
    din("bmask", [C, C], f32)
    din("ident", [128, 128], bf16)
    din("dwA", [128, len(_NON_PE)], f32)
    din("dwB", [128, len(_NON_PE)], f32)
    y_d = nc.dram_tensor("y", [C, N], bf16, kind="ExternalOutput").ap()

    dma = nc.sync.dma_start
    V, S, P, T = nc.vector, nc.scalar, nc.gpsimd, nc.tensor
    mult, add = AluOpType.mult, AluOpType.add

    def MM(out, lhsT, rhs, start, stop):
        T.matmul(out, lhsT, rhs, start=start, stop=stop, skip_group_check=True)

    stack = ExitStack()
    with stack:
        # ================= persistent weights =================
        wpool = stack.enter_context(tc.tile_pool(name="weights", bufs=1))

        def w2(nm, dty=bf16, src=None):
            src = dram[src or nm]
            t0 = wpool.tile([128, src.shape[1]], dty, name=f"{nm}0", tag=f"{nm}0")
            t1 = wpool.tile([64, src.shape[1]], dty, name=f"{nm}1", tag=f"{nm}1")
            dma(t0[:, :], src[0:128, :])
            dma(t1[:, :], src[128:C, :])
            return t0, t1

        wqT = w2("wqT"); wkT = w2("wkT"); wvT = w2("wvT")

        def w2dup(nm):
            """chunk0 [128,C]; chunk1 duplicated into lanes 0:64 and 64:128
            so rhs views based at partition 64 have an aligned lhsT."""
            src = dram[nm]
            t0 = wpool.tile([128, C], bf16, name=f"{nm}0", tag=f"{nm}0")
            dma(t0[:, :], src[0:128, :])
            t1 = wpool.tile([128, C], bf16, name=f"{nm}1", tag=f"{nm}1")
            dma(t1[0:64, :], src[128:C, :])
            dma(t1[64:128, :], src[128:C, :])
            return t0, t1

        wpT = w2dup("wpT")
        wps = [w2dup(f"wps{i}T") for i in range(len(PE_TAPS))]

        def w96(nm):
            ts = []
            for i in range(2):
                tt = wpool.tile([96, C], bf16, name=f"{nm}_{i}", tag=f"{nm}_{i}")
                dma(tt[:, :], dram[nm][i * 96:(i + 1) * 96, :])
                ts.append(tt)
            return ts

        wqn = w96("wq_nat"); wkn = w96("wk_nat")
        bmask = []
        for i in range(2):
            bm = wpool.tile([96, C], f32, name=f"bmask{i}", tag=f"bmask{i}")
            dma(bm[:, :], dram["bmask"][i * 96:(i + 1) * 96, :])
            bmask.append(bm)
        temp96 = []
        for i in range(2):
            tt = wpool.tile([96, 1], f32, name=f"temp{i}", tag=f"temp{i}")
            dma(tt[:, :], dram["temp_pc"][i * 96:(i + 1) * 96, :])
            temp96.append(tt)
        ident = wpool.tile([128, 128], bf16, name="ident", tag="ident")
        dma(ident[:, :], dram["ident"][:, :])
        dwA = wpool.tile([128, len(_NON_PE)], f32, name="dwA", tag="dwA")
        dma(dwA[:, :], dram["dwA"][:, :])
        dwB = wpool.tile([128, len(_NON_PE)], f32, name="dwB", tag="dwB")
        dma(dwB[:, :], dram["dwB"][:, :])

        # v stays resident until t is built
        vpool = stack.enter_context(tc.tile_pool(name="vres", bufs=1))
        v96 = [vpool.tile([96, N], bf16, name=f"v{i}", tag=f"v{i}") for i in range(2)]
        # x stays resident until v is built (v matmuls run in phase 3);
        # freed before the stripe pool opens (LIFO pool order holds).
        xstack = ExitStack()
        xpool = xstack.enter_context(tc.tile_pool(name="xres", bufs=1))
        xA = xpool.tile([128, N], bf16, name="xA", tag="xA")
        xB = xpool.tile([64, N], bf16, name="xB", tag="xB")

        # ====== phase 1+2: load, cast, transpose (PE), Gram ======
        ph2 = ExitStack()
        with ph2:
            pf32 = ph2.enter_context(tc.tile_pool(name="xf32", bufs=3))
            pxT = ph2.enter_context(tc.tile_pool(name="xT", bufs=3))
            ptr = ph2.enter_context(
                tc.tile_pool(name="trps", bufs=4, space="PSUM"))
            pXps = ph2.enter_context(
                tc.tile_pool(name="Xps", bufs=1, space="PSUM"))
            psum_XA = pXps.tile([128, C], f32, name="psXA", tag="psXA")
            psum_XB = pXps.tile([64, C], f32, name="psXB", tag="psXB")

            for g in range(NGX):
                px = g * GPX
                fA = pf32.tile([128, GPX], f32, tag="fA")
                dma(fA[:, :], dram["x"][0:128, px:px + GPX])
                fB = pf32.tile([64, GPX], f32, tag="fB")
                dma(fB[:, :], dram["x"][128:C, px:px + GPX])
                V.tensor_copy(xA[:, px:px + GPX], fA[:, :])
                S.copy(xB[:, px:px + GPX], fB[:, :])

                # PE transpose per 128-pixel chunk into one shared psum tile,
                # single evac (alternating DVE/ACT), then Gram accumulation.
                xT = pxT.tile([128, XG * C], bf16, tag="xT")
                for i in range(XG):
                    ch = g * XG + i
                    s0 = px + i * 128
                    ps = ptr.tile([128, C], bf16, tag="tr")
                    T.transpose(ps[:, 0:128], xA[:, s0:s0 + 128], ident[:, :])
                    T.transpose(ps[:, 128:C], xB[:, s0:s0 + 128],
                                ident[0:64, 0:64])
                    dst = xT[:, i * C:(i + 1) * C]
                    if ch % 2 == 0:
                        V.tensor_copy(dst, ps[:, :])
                    else:
                        S.copy(dst, ps[:, :])
                for i in range(XG):
                    ch = g * XG + i
                    first, last = ch == 0, ch == NCH - 1
                    rhs = xT[:, i * C:(i + 1) * C]
                    MM(psum_XA[:, :], xT[:, i * C:i * C + 128], rhs, first, last)
                    MM(psum_XB[:, :], xT[:, i * C + 128:(i + 1) * C], rhs,
                       first, last)

            Xb = (wpool.tile([128, C], bf16, name="Xb0", tag="Xb0"),
                  wpool.tile([64, C], bf16, name="Xb1", tag="Xb1"))
            S.copy(Xb[0][:, :], psum_XA[:, :])
            S.copy(Xb[1][:, :], psum_XB[:, :])

        # ================= phase 3: tiny attention chain =================
        ph3 = ExitStack()
        with ph3:
            p3s = ph3.enter_context(tc.tile_pool(name="tiny", bufs=1))
            p3p = ph3.enter_context(
                tc.tile_pool(name="tinyps", bufs=1, space="PSUM"))

            def rowdot_norms(wT, wn):
                """returns [rinv0, rinv1] tiles [96,1] f32 = 1/max(|row|,eps)"""
                outs = []
                for mc in range(2):
                    msl = slice(mc * 96, (mc + 1) * 96)
                    ps = p3p.tile([96, C], f32, tag="aq")
                    MM(ps[:, :], wT[0][:, msl], Xb[0][:, :], True, False)
                    MM(ps[:, :], wT[1][:, msl], Xb[1][:, :], False, True)
                    prod = p3s.tile([96, C], f32, name=f"prod{mc}", tag=f"prod{mc}")
                    V.tensor_tensor(prod[:, :], ps[:, :], wn[mc][:, :], mult)
                    n2 = p3s.tile([96, 1], f32, name=f"n2_{mc}", tag=f"n2_{mc}")
                    V.tensor_reduce(n2[:, :], prod[:, :], AX.X, AluOpType.add)
                    V.tensor_scalar_max(n2[:, :], n2[:, :], EPS * EPS)
                    sq = p3s.tile([96, 1], f32, name=f"sq{mc}", tag=f"sq{mc}")
                    S.sqrt(sq[:, :], n2[:, :])
                    rv = p3s.tile([96, 1], f32, name=f"rv{mc}", tag=f"rv{mc}")
                    V.reciprocal(rv[:, :], sq[:, :])
                    outs.append(rv)
                return outs

            rq = rowdot_norms(wqT, wqn)
            rk = rowdot_norms(wkT, wkn)
            for mc in range(2):
                V.tensor_tensor(rq[mc][:, :], rq[mc][:, :], temp96[mc][:, :], mult)

            # normalized+scaled weights, then transpose on PE
            wqs, wks = [], []
            for mc in range(2):
                a = p3s.tile([96, C], bf16, name=f"wqs{mc}", tag=f"wqs{mc}")
                V.tensor_scalar_mul(a[:, :], wqn[mc][:, :], rq[mc][:, :])
                wqs.append(a)
                b = p3s.tile([96, C], bf16, name=f"wks{mc}", tag=f"wks{mc}")
                V.tensor_scalar_mul(b[:, :], wkn[mc][:, :], rk[mc][:, :])
                wks.append(b)

            def transpose_pair(src_pair, nm):
                """[96,C]x2 (rows m, cols c) -> c-chunked pair [128,192],[64,192]"""
                d0 = p3s.tile([128, C], bf16, name=f"{nm}0", tag=f"{nm}0")
                d1 = p3s.tile([64, C], bf16, name=f"{nm}1", tag=f"{nm}1")
                for mc in range(2):
                    for cc, (c0, csz, dst) in enumerate(
                            [(0, 128, d0), (128, 64, d1)]):
                        ps = p3p.tile([csz, 96], bf16, tag=f"trp{cc}")
                        T.transpose(ps[:, :], src_pair[mc][:, c0:c0 + csz],
                                    ident[0:96, 0:96])
                        S.copy(dst[:, mc * 96:(mc + 1) * 96], ps[:, :])
                return d0, d1

            wqsT = transpose_pair(wqs, "wqsT")
            wksT = transpose_pair(wks, "wksT")

            # AkT = X @ Wkn^T  (X symmetric)
            akT0 = p3s.tile([128, C], bf16, name="akT0", tag="akT0")
            akT1 = p3s.tile([64, C], bf16, name="akT1", tag="akT1")
            for (m0, msz, dst) in [(0, 128, akT0), (128, 64, akT1)]:
                ps = p3p.tile([msz, C], f32, tag="akTps")
                MM(ps[:, :], Xb[0][:, m0:m0 + msz], wksT[0][:, :], True, False)
                MM(ps[:, :], Xb[1][:, m0:m0 + msz], wksT[1][:, :], False, True)
                S.copy(dst[:, :], ps[:, :])

            # G = Wqn @ AkT ; masked full-row softmax (mask = -3e4 off own
            # head's 12x12 block -> exact zeros after exp) gives the
            # block-diagonal attention matrix rows directly.
            abdT = []
            for mc in range(2):
                msl = slice(mc * 96, (mc + 1) * 96)
                psG = p3p.tile([96, C], f32, tag="psG")
                MM(psG[:, :], wqsT[0][:, msl], akT0[:, :], True, False)
                MM(psG[:, :], wqsT[1][:, msl], akT1[:, :], False, True)
                gf = p3s.tile([96, C], f32, name=f"gf{mc}", tag=f"gf{mc}")
                V.tensor_tensor(gf[:, :], psG[:, :], bmask[mc][:, :], add)
                mx = p3s.tile([96, 1], f32, name=f"mx{mc}", tag=f"mx{mc}")
                V.tensor_reduce(mx[:, :], gf[:, :], AX.X, AluOpType.max)
                V.tensor_scalar_mul(mx[:, :], mx[:, :], -1.0)
                ex = p3s.tile([96, C], f32, name=f"ex{mc}", tag=f"ex{mc}")
                S.activation(ex[:, :], gf[:, :], AF.Exp, bias=mx[:, :])
                sm = p3s.tile([96, 1], f32, name=f"sm{mc}", tag=f"sm{mc}")
                V.tensor_reduce(sm[:, :], ex[:, :], AX.X, AluOpType.add)
                V.reciprocal(sm[:, :], sm[:, :])
                at = p3s.tile([96, C], bf16, name=f"at{mc}", tag=f"at{mc}")
                V.tensor_scalar_mul(at[:, :], ex[:, :], sm[:, :])
                pst = p3p.tile([96, 96], bf16, tag="attnT")
                T.transpose(pst[:, :], at[:, mc * 96:(mc + 1) * 96],
                            ident[0:96, 0:96])
                ab = vpool.tile([96, C], bf16, name=f"abdT{mc}", tag=f"abdT{mc}")
                V.memset(ab[:, :], 0)
                S.copy(ab[:, mc * 96:(mc + 1) * 96], pst[:, :])
                abdT.append(ab)

            # ---- v = Wv @ x (PE-filler during the latency-bound chain) ----
            pvps = ph3.enter_context(
                tc.tile_pool(name="vps", bufs=2, space="PSUM"))
            for j in range(NJT):
                sl = slice(j * PT, (j + 1) * PT)
                for mc in range(2):
                    pv = pvps.tile([96, PT], f32, tag="pv")
                    msl = slice(mc * 96, (mc + 1) * 96)
                    MM(pv[:, :], wvT[0][:, msl], xA[:, sl], True, False)
                    MM(pv[:, :], wvT[1][:, msl], xB[:, sl], False, True)
                    S.copy(v96[mc][:, sl], pv[:, :])
        xstack.close()

        # ================= phase 4+5: t, depthwise, proj =================
        ph5 = ExitStack()
        with ph5:
            pstr = ph5.enter_context(tc.tile_pool(name="stripes", bufs=1))
            tA = [pstr.tile([128, SFREE], bf16, name=f"tA{i}", tag=f"tA{i}")
                  for i in range(2)]
            tB = pstr.tile([128, SFREE], bf16, name="tB", tag="tB")
            ptps = ph5.enter_context(
                tc.tile_pool(name="tps", bufs=2, space="PSUM"))
            pyps = ph5.enter_context(
                tc.tile_pool(name="yps", bufs=2, space="PSUM"))
            pz = ph5.enter_context(tc.tile_pool(name="z", bufs=3))
            pzB = ph5.enter_context(tc.tile_pool(name="zB", bufs=NDT + 1))
            ptmp = ph5.enter_context(tc.tile_pool(name="dwtmp", bufs=4))
            pys = ph5.enter_context(tc.tile_pool(name="ystage", bufs=3))

            def st3(tile_):
                return tile_.rearrange("p (r c) -> p r c", c=TPW)

            # border zeros
            for st in tA:
                P.memset(st3(st)[:, 0:1, :], 0)         # top pad row
                P.memset(st3(st)[:, SROWS - 1:SROWS, :], 0)  # bottom pad row
            P.memset(st3(tB)[0:64, 0:1, :], 0)
            P.memset(st3(tB)[64:128, SROWS - 1:SROWS, :], 0)
            # the "real data" edge rows of each stripe get their pad row
            # overwritten by the dup-row evacs below except at the extremes;
            # memset all four stripe edge rows then col borders:
            P.memset(st3(tB)[0:64, SROWS - 1:SROWS, :], 0)
            P.memset(st3(tB)[64:128, 0:1, :], 0)
            for st in (tA[0], tA[1], tB):
                P.memset(st3(st)[:, :, 0:1], 0)
                P.memset(st3(st)[:, :, TPW - 1:TPW], 0)

            # ---- t = abdT.T @ v, evacuated into padded stripes ----
            for j in range(NJT):
                half, jl = divmod(j, HJT)
                px = j * PT
                sl = slice(px, px + PT)
                stA = tA[half]
                psA = ptps.tile([128, PT], f32, tag="ptA")
                MM(psA[:, :], abdT[0][:, 0:128], v96[0][:, sl], True, False)
                MM(psA[:, :], abdT[1][:, 0:128], v96[1][:, sl], False, True)
                psB = ptps.tile([128, PT], f32, tag="ptB")
                bsl = slice(0, 64) if half == 0 else slice(64, 128)
                MM(psB[bsl, :], abdT[1][:, 128:C], v96[1][:, sl], True, True)

                r0 = jl * PR + 1  # local padded row of first row
                S.copy(st3(stA)[:, r0:r0 + PR, 1:1 + W_], psA[:, :])
                S.copy(st3(tB)[bsl, r0:r0 + PR, 1:1 + W_], psB[bsl, :])

                if j == HJT - 1:  # rows halfH-1: dup into bottom stripes row 0
                    S.copy(st3(tA[1])[:, 0:1, 1:1 + W_], psA[:, PT - W_:PT])
                    psBx = ptps.tile([128, PT], f32, tag="ptB")
                    MM(psBx[64:128, :], abdT[1][:, 128:C], v96[1][:, sl],
                       True, True)
                    S.copy(st3(tB)[64:128, 0:1, 1:1 + W_], psBx[64:128, PT - W_:PT])
                if j == HJT:  # row halfH: dup into top stripes last row
                    S.copy(st3(tA[0])[:, SROWS - 1:SROWS, 1:1 + W_],
                           psA[:, 0:W_])
                    psBx = ptps.tile([128, PT], f32, tag="ptB")
                    MM(psBx[0:64, :], abdT[1][:, 128:C], v96[1][:, sl],
                       True, True)
                    S.copy(st3(tB)[0:64, SROWS - 1:SROWS, 1:1 + W_],
                           psBx[0:64, 0:W_])

            # ---- depthwise + projection ----
            iACT = [_NON_PE.index(t_) for t_ in ACT_TAPS]
            iDVE = [_NON_PE.index(t_) for t_ in DVE_TAPS]

            def dw_tile(stripe, lanes, jd, dw, ztile):
                """emit the 7 non-PE taps for RT out-rows into ztile[lanes]"""
                s3 = st3(stripe)
                z3 = ztile.rearrange("p (r c) -> p r c", c=W_)

                def vw(k):
                    di, dj = _NON_PE[k]
                    return s3[lanes, jd * RT + di:jd * RT + di + RT, dj:dj + W_]

                zv = z3[lanes, :, :]
                k0 = iDVE[0]
                V.tensor_scalar_mul(zv, vw(k0), dw[lanes, k0:k0 + 1])
                for k in iDVE[1:]:
                    tm = ptmp.tile([128, RT * W_], bf16, tag="tmp")
                    t3 = tm.rearrange("p (r c) -> p r c", c=W_)[lanes, :, :]
                    V.tensor_scalar_mul(t3, vw(k), dw[lanes, k:k + 1])
                    V.tensor_tensor(zv, zv, t3, add)
                for k, eng in zip(iACT, ACT_MERGE):
                    tm = ptmp.tile([128, RT * W_], bf16, tag="tmp")
                    t3 = tm.rearrange("p (r c) -> p r c", c=W_)[lanes, :, :]
                    S.activation(t3, vw(k), AF.Copy, scale=dw[lanes, k:k + 1])
                    (P if eng == "pool" else V).tensor_tensor(zv, zv, t3, add)

            allL = slice(0, 128)
            for half in range(2):
                zB_tiles = {}
                for jt in range(HJT):
                    jd, part = divmod(jt, RPT)
                    j = half * HJT + jt
                    if part == 0:
                        zA = pz.tile([128, RT * W_], bf16, tag="zA")
                        dw_tile(tA[half], allL, jd, dwA, zA)
                        if half == 0:
                            zB = pzB.tile([128, RT * W_], bf16, tag="zB")
                            dw_tile(tB, allL, jd, dwB, zB)
                            zB_tiles[jd] = zB
                        else:
                            zB = zB_tiles_prev[jd]
                    zsl = slice(part * PT, (part + 1) * PT)
                    bsl = slice(0, 64) if half == 0 else slice(64, 128)
                    r0 = jt * PR  # local padded row base for PE tap views
                    ps_pair = []
                    for (m0, msz, pst) in [(0, 128, None), (128, 64, None)]:
                        psy = pyps.tile([msz, PT], f32, tag=f"py{m0}")
                        msl = slice(m0, m0 + msz)
                        MM(psy[:, :], wpT[0][:, msl], zA[:, zsl], True, False)
                        MM(psy[:, :], wpT[1][bsl, msl], zB[bsl, zsl],
                           False, False)
                        for i, (di, dj) in enumerate(PE_TAPS):
                            vA = st3(tA[half])[:, r0 + di:r0 + di + PR,
                                               dj:dj + W_]
                            vB = st3(tB)[bsl, r0 + di:r0 + di + PR, dj:dj + W_]
                            MM(psy[:, :], wps[i][0][:, msl], vA, False, False)
                            MM(psy[:, :], wps[i][1][bsl, msl], vB, False,
                               i == len(PE_TAPS) - 1)
                        ys = pys.tile([msz, PT], bf16, tag=f"ys{m0}")
                        S.copy(ys[:, :], psy[:, :])
                        dma(y_d[m0:m0 + msz, j * PT:(j + 1) * PT], ys[:, :])
                if half == 0:
                    zB_tiles_prev = zB_tiles
    return dram, y_d


# --------------------------------------------------------------------------
# host entry
# --------------------------------------------------------------------------

_CACHE = {}


def build_module(plan, num_devices=8):
    import concourse.bacc as bacc
    import concourse.tile as tile

    nc = bacc.Bacc("TRN2", target_bir_lowering=False, debug=False,
                   num_devices=num_devices)
    with tile.TileContext(nc) as tc:
        build_graph(nc, tc, plan)
    nc.compile()
    return nc


def _build_and_run(in_maps, plan):
    from concourse import bass_utils

    key = (plan["H"], plan["W"])
    if key not in _CACHE:
        _CACHE[key] = build_module(plan, num_devices=len(in_maps))
    nc = _CACHE[key]
    res = bass_utils.run_bass_kernel_spmd(
        nc, in_maps, core_ids=list(range(len(in_maps))))
    return res


def kernel(x, w_qkv, w_dw, w_proj, temperature):
    x = np.asarray(x, np.float32)
    plan = make_plan(H, W)
    prep = host_prep(w_qkv, w_dw, w_proj, temperature)
    xf = x.reshape(B, C, H * W)
    in_maps = [{"x": np.ascontiguousarray(xf[b]), **prep} for b in range(B)]
    res = _build_and_run(in_maps, plan)
    y = np.stack([np.asarray(r["y"], np.float32) for r in res.results])
    return y.reshape(B, C, H, W).astype(np.float32)


# revision 27
# speedup vs baseline: 488.0490x; 488.0490x over previous
"""Channel-attention block (QKV 1x1 -> L2-normalized channel attention ->
depthwise 3x3 -> 1x1 proj) on 8 Trainium2 NeuronCores, data-parallel over
the batch (1 image per core).

Exact algebraic refactoring of the reference, per image x [C, N]:
    X   = x @ x.T                          (Gram; replaces the whole QKV)
    Aq  = Wq @ X ; nq2 = rowdot(Aq, Wq)    = diag(Wq X Wq^T) = |q_row|^2
    Ak  = Wk @ X ; nk2 = rowdot(Ak, Wk)
    rq  = temp / max(sqrt(nq2), eps) ; rk = 1 / max(sqrt(nk2), eps)
    G   = (diag(rq) Wq) X (diag(rk) Wk)^T  = attention logits
    attn = row-softmax of per-head 12x12 diagonal blocks of G
    v   = Wv @ x
    t   = blockdiag(attn) @ v
    z   = depthwise3x3(t);  y = Wp @ z

The depthwise 3x3 (per-channel scale x shifted view, 9 taps) is split
across engines: 2 taps folded into the projection matmul on PE (lhsT
pre-scaled by the tap weight, shifted rhs view, PSUM accumulates), 2
fused mul+adds on GpSimd, 2 scaled copies on ScalarE (merged by DVE),
3 on VectorE.

Channel layout: C=192 > 128 partitions, so channel-dim tensors are chunk
pairs [128,*]+[64,*] (or [96,*]x2 where head alignment matters). The
padded t for the depthwise conv is stored as three "stripes" of
[128, (H/2+2)*(W+2)] bf16: channels 0-127 top half, channels 0-127
bottom half, and channels 128-191 dual-packed (lanes 0-63 top half,
lanes 64-127 bottom half) so elementwise engines never run half-empty.

Self-contained: full unsharded inputs in, full output out.
"""

import numpy as np
from contextlib import ExitStack

B, C, H, W = 8, 192, 128, 128
HEADS = 16
HD = C // HEADS  # 12
EPS = 1e-12

ALL_TAPS = [(di, dj) for di in range(3) for dj in range(3)]
PE_TAPS = [(0, 1), (2, 1)]
# ACT computes scaled copies; merge engine per ACT tap listed alongside.
ACT_TAPS = [(1, 1), (2, 0), (0, 0)]
ACT_MERGE = ["pool", "dve", "dve"]
DVE_TAPS = [(1, 0), (0, 2), (1, 2), (2, 2)]  # first is the z-init mul
_NON_PE = ACT_TAPS + DVE_TAPS


def _bf16_np():
    import ml_dtypes
    return ml_dtypes.bfloat16


def make_plan(H_, W_):
    N = H_ * W_
    halfH = H_ // 2
    TPW = W_ + 2
    PR = max(1, min(512 // W_, halfH))
    while halfH % PR:
        PR -= 1
    RT = 2 * PR if halfH % (2 * PR) == 0 else PR
    NCH = N // 128
    assert N % 128 == 0
    XG = max(1, NCH // 8)
    while NCH % XG:
        XG -= 1
    return dict(H=H_, W=W_, N=N, halfH=halfH, TPW=TPW, PR=PR, RT=RT,
                NCH=NCH, XG=XG, SROWS=halfH + 2)


def host_prep(w_qkv, w_dw, w_proj, temperature):
    bf16 = _bf16_np()
    w_qkv = np.asarray(w_qkv, np.float32)
    wdw = np.asarray(w_dw, np.float32).reshape(C, 3, 3)
    w_proj = np.asarray(w_proj, np.float32)
    temp = np.asarray(temperature, np.float32).reshape(HEADS)

    Wq, Wk, Wv = w_qkv[:C], w_qkv[C:2 * C], w_qkv[2 * C:]
    ins = {
        "wqT": np.ascontiguousarray(Wq.T).astype(bf16),
        "wkT": np.ascontiguousarray(Wk.T).astype(bf16),
        "wv_nat": Wv.astype(bf16),
        "wq_nat": Wq.astype(bf16),
        "wk_nat": Wk.astype(bf16),
        "temp_pc": np.repeat(temp, HD).reshape(C, 1).astype(np.float32),
        "ident": np.eye(128, dtype=np.float32).astype(bf16),
        "wpT": np.ascontiguousarray(w_proj.T).astype(bf16),
        "bmask": np.where(np.arange(C)[:, None] // HD == np.arange(C)[None] // HD,
                          0.0, -3e4).astype(np.float32),
    }
    for i, (di, dj) in enumerate(PE_TAPS):
        ins[f"wps{i}T"] = np.ascontiguousarray(
            w_proj.T * wdw[:, di, dj][:, None]).astype(bf16)
    ntap = len(_NON_PE)
    dwA = np.zeros((128, ntap), np.float32)
    dwB = np.zeros((128, ntap), np.float32)
    for k, (di, dj) in enumerate(_NON_PE):
        dwA[:, k] = wdw[:128, di, dj]
        dwB[:64, k] = wdw[128:, di, dj]
        dwB[64:, k] = wdw[128:, di, dj]
    ins["dwA"] = dwA
    ins["dwB"] = dwB
    return ins


def numpy_fold(x_img, w_qkv, w_dw, w_proj, temperature, plan):
    """Golden fp32 numpy model of the folded algorithm."""
    H_, W_, N = plan["H"], plan["W"], plan["N"]
    w_qkv = np.asarray(w_qkv, np.float32)
    wdw = np.asarray(w_dw, np.float32).reshape(C, 3, 3)
    Wp = np.asarray(w_proj, np.float32)
    temp = np.asarray(temperature, np.float32).reshape(HEADS)
    Wq, Wk, Wv = w_qkv[:C], w_qkv[C:2 * C], w_qkv[2 * C:]
    xf = np.asarray(x_img, np.float32).reshape(C, N)
    X = xf @ xf.T
    nq2 = ((Wq @ X) * Wq).sum(1)
    nk2 = ((Wk @ X) * Wk).sum(1)
    rq = np.repeat(temp, HD) / np.maximum(np.sqrt(np.maximum(nq2, EPS * EPS)), EPS)
    rk = 1.0 / np.maximum(np.sqrt(np.maximum(nk2, EPS * EPS)), EPS)
    G = (rq[:, None] * Wq) @ X @ (rk[:, None] * Wk).T
    A_bd = np.zeros((C, C), np.float32)
    for h in range(HEADS):
        s = slice(h * HD, (h + 1) * HD)
        g = G[s, s]
        g = g - g.max(1, keepdims=True)
        e = np.exp(g)
        A_bd[s, s] = e / e.sum(1, keepdims=True)
    t = (A_bd @ (Wv @ xf)).reshape(C, H_, W_)
    tp = np.pad(t, ((0, 0), (1, 1), (1, 1)))
    z = np.zeros_like(t)
    for di in range(3):
        for dj in range(3):
            z += wdw[:, di, dj][:, None, None] * tp[:, di:di + H_, dj:dj + W_]
    return (Wp @ z.reshape(C, N)).reshape(C, H_, W_)


# --------------------------------------------------------------------------
# device graph
# --------------------------------------------------------------------------

def build_graph(nc, tc, plan):
    import concourse.mybir as mybir
    from concourse.alu_op_type import AluOpType
    dt = mybir.dt
    AF = mybir.ActivationFunctionType
    AX = mybir.AxisListType
    f32, bf16 = dt.float32, dt.bfloat16

    H_, W_, N = plan["H"], plan["W"], plan["N"]
    halfH, TPW, PR, RT = plan["halfH"], plan["TPW"], plan["PR"], plan["RT"]
    NCH, XG, SROWS = plan["NCH"], plan["XG"], plan["SROWS"]
    PT = PR * W_
    NJT = N // PT
    HJT = NJT // 2
    NDT = halfH // RT
    RPT = RT // PR
    NGX = NCH // XG
    GPX = XG * 128          # pixels per load group
    SFREE = SROWS * TPW

    # ---- DRAM ----
    dram = {}
    def din(name, shape, dty):
        dram[name] = nc.dram_tensor(name, shape, dty, kind="ExternalInput").ap()
    din("x", [C, N], f32)
    for nm in ["wqT", "wkT", "wv_nat", "wq_nat", "wk_nat", "wpT"]:
        din(nm, [C, C], bf16)
    for i in range(len(PE_TAPS)):
        din(f"wps{i}T", [C, C], bf16)
    din("temp_pc", [C, 1], f32)
    din("bmask", [C, C], f32)
    din("ident", [128, 128], bf16)
    din("dwA", [128, len(_NON_PE)], f32)
    din("dwB", [128, len(_NON_PE)], f32)
    y_d = nc.dram_tensor("y", [C, N], bf16, kind="ExternalOutput").ap()

    dma = nc.sync.dma_start
    V, S, P, T = nc.vector, nc.scalar, nc.gpsimd, nc.tensor
    mult, add = AluOpType.mult, AluOpType.add

    def MM(out, lhsT, rhs, start, stop):
        T.matmul(out, lhsT, rhs, start=start, stop=stop, skip_group_check=True)

    stack = ExitStack()
    with stack:
        # ================= persistent weights =================
        wpool = stack.enter_context(tc.tile_pool(name="weights", bufs=1))

        def w2(nm, dty=bf16, src=None):
            src = dram[src or nm]
            t0 = wpool.tile([128, src.shape[1]], dty, name=f"{nm}0", tag=f"{nm}0")
            t1 = wpool.tile([64, src.shape[1]], dty, name=f"{nm}1", tag=f"{nm}1")
            dma(t0[:, :], src[0:128, :])
            dma(t1[:, :], src[128:C, :])
            return t0, t1

        wqT = w2("wqT"); wkT = w2("wkT")

        def w2dup(nm):
            """chunk0 [128,C]; chunk1 duplicated into lanes 0:64 and 64:128
            so rhs views based at partition 64 have an aligned lhsT."""
            src = dram[nm]
            t0 = wpool.tile([128, C], bf16, name=f"{nm}0", tag=f"{nm}0")
            dma(t0[:, :], src[0:128, :])
            t1 = wpool.tile([128, C], bf16, name=f"{nm}1", tag=f"{nm}1")
            dma(t1[0:64, :], src[128:C, :])
            dma(t1[64:128, :], src[128:C, :])
            return t0, t1

        wpT = w2dup("wpT")
        wps = [w2dup(f"wps{i}T") for i in range(len(PE_TAPS))]

        def w96(nm):
            ts = []
            for i in range(2):
                tt = wpool.tile([96, C], bf16, name=f"{nm}_{i}", tag=f"{nm}_{i}")
                dma(tt[:, :], dram[nm][i * 96:(i + 1) * 96, :])
                ts.append(tt)
            return ts

        wqn = w96("wq_nat"); wkn = w96("wk_nat"); wvn = w96("wv_nat")
        bmask = []
        for i in range(2):
            bm = wpool.tile([96, C], f32, name=f"bmask{i}", tag=f"bmask{i}")
            dma(bm[:, :], dram["bmask"][i * 96:(i + 1) * 96, :])
            bmask.append(bm)
        temp96 = []
        for i in range(2):
            tt = wpool.tile([96, 1], f32, name=f"temp{i}", tag=f"temp{i}")
            dma(tt[:, :], dram["temp_pc"][i * 96:(i + 1) * 96, :])
            temp96.append(tt)
        ident = wpool.tile([128, 128], bf16, name="ident", tag="ident")
        dma(ident[:, :], dram["ident"][:, :])
        dwA = wpool.tile([128, len(_NON_PE)], f32, name="dwA", tag="dwA")
        dma(dwA[:, :], dram["dwA"][:, :])
        dwB = wpool.tile([128, len(_NON_PE)], f32, name="dwB", tag="dwB")
        dma(dwB[:, :], dram["dwB"][:, :])

        # holds abdT / W'vT between phase 3 and the t matmuls
        vpool = stack.enter_context(tc.tile_pool(name="vres", bufs=1))
        # x stays resident until t is built (t = W'v @ x in phase 4)
        xpool = stack.enter_context(tc.tile_pool(name="xres", bufs=1))
        xA = xpool.tile([128, N], bf16, name="xA", tag="xA")
        xB = xpool.tile([64, N], bf16, name="xB", tag="xB")

        # ====== phase 1+2: load, cast, transpose (PE), Gram ======
        ph2 = ExitStack()
        with ph2:
            pf32 = ph2.enter_context(tc.tile_pool(name="xf32", bufs=3))
            pxT = ph2.enter_context(tc.tile_pool(name="xT", bufs=3))
            ptr = ph2.enter_context(
                tc.tile_pool(name="trps", bufs=4, space="PSUM"))
            pXps = ph2.enter_context(
                tc.tile_pool(name="Xps", bufs=1, space="PSUM"))
            psum_XA = pXps.tile([128, C], f32, name="psXA", tag="psXA")
            psum_XB = pXps.tile([64, C], f32, name="psXB", tag="psXB")

            for g in range(NGX):
                px = g * GPX
                fA = pf32.tile([128, GPX], f32, tag="fA")
                dma(fA[:, :], dram["x"][0:128, px:px + GPX])
                fB = pf32.tile([64, GPX], f32, tag="fB")
                dma(fB[:, :], dram["x"][128:C, px:px + GPX])
                V.tensor_copy(xA[:, px:px + GPX], fA[:, :])
                S.copy(xB[:, px:px + GPX], fB[:, :])

                # PE transpose per 128-pixel chunk into one shared psum tile,
                # single evac (alternating DVE/ACT), then Gram accumulation.
                xT = pxT.tile([128, XG * C], bf16, tag="xT")
                for i in range(XG):
                    ch = g * XG + i
                    s0 = px + i * 128
                    ps = ptr.tile([128, C], bf16, tag="tr")
                    T.transpose(ps[:, 0:128], xA[:, s0:s0 + 128], ident[:, :])
                    T.transpose(ps[:, 128:C], xB[:, s0:s0 + 128],
                                ident[0:64, 0:64])
                    dst = xT[:, i * C:(i + 1) * C]
                    if ch % 2 == 0:
                        V.tensor_copy(dst, ps[:, :])
                    else:
                        S.copy(dst, ps[:, :])
                for i in range(XG):
                    ch = g * XG + i
                    first, last = ch == 0, ch == NCH - 1
                    rhs = xT[:, i * C:(i + 1) * C]
                    MM(psum_XA[:, :], xT[:, i * C:i * C + 128], rhs, first, last)
                    MM(psum_XB[:, :], xT[:, i * C + 128:(i + 1) * C], rhs,
                       first, last)

            Xb = (wpool.tile([128, C], bf16, name="Xb0", tag="Xb0"),
                  wpool.tile([64, C], bf16, name="Xb1", tag="Xb1"))
            S.copy(Xb[0][:, :], psum_XA[:, :])
            S.copy(Xb[1][:, :], psum_XB[:, :])

        # ================= phase 3: tiny attention chain =================
        ph3 = ExitStack()
        with ph3:
            p3s = ph3.enter_context(tc.tile_pool(name="tiny", bufs=1))
            p3p = ph3.enter_context(
                tc.tile_pool(name="tinyps", bufs=1, space="PSUM"))

            def rowdot_norms(wT, wn):
                """returns [rinv0, rinv1] tiles [96,1] f32 = 1/max(|row|,eps)"""
                outs = []
                for mc in range(2):
                    msl = slice(mc * 96, (mc + 1) * 96)
                    ps = p3p.tile([96, C], f32, tag="aq")
                    MM(ps[:, :], wT[0][:, msl], Xb[0][:, :], True, False)
                    MM(ps[:, :], wT[1][:, msl], Xb[1][:, :], False, True)
                    prod = p3s.tile([96, C], f32, name=f"prod{mc}", tag=f"prod{mc}")
                    V.tensor_tensor(prod[:, :], ps[:, :], wn[mc][:, :], mult)
                    n2 = p3s.tile([96, 1], f32, name=f"n2_{mc}", tag=f"n2_{mc}")
                    V.tensor_reduce(n2[:, :], prod[:, :], AX.X, AluOpType.add)
                    V.tensor_scalar_max(n2[:, :], n2[:, :], EPS * EPS)
                    sq = p3s.tile([96, 1], f32, name=f"sq{mc}", tag=f"sq{mc}")
                    S.sqrt(sq[:, :], n2[:, :])
                    rv = p3s.tile([96, 1], f32, name=f"rv{mc}", tag=f"rv{mc}")
                    V.reciprocal(rv[:, :], sq[:, :])
                    outs.append(rv)
                return outs

            rq = rowdot_norms(wqT, wqn)
            rk = rowdot_norms(wkT, wkn)
            for mc in range(2):
                V.tensor_tensor(rq[mc][:, :], rq[mc][:, :], temp96[mc][:, :], mult)

            # normalized+scaled weights, then transpose on PE
            wqs, wks = [], []
            for mc in range(2):
                a = p3s.tile([96, C], bf16, name=f"wqs{mc}", tag=f"wqs{mc}")
                V.tensor_scalar_mul(a[:, :], wqn[mc][:, :], rq[mc][:, :])
                wqs.append(a)
                b = p3s.tile([96, C], bf16, name=f"wks{mc}", tag=f"wks{mc}")
                V.tensor_scalar_mul(b[:, :], wkn[mc][:, :], rk[mc][:, :])
                wks.append(b)

            def transpose_pair(src_pair, nm):
                """[96,C]x2 (rows m, cols c) -> c-chunked pair [128,192],[64,192]"""
                d0 = p3s.tile([128, C], bf16, name=f"{nm}0", tag=f"{nm}0")
                d1 = p3s.tile([64, C], bf16, name=f"{nm}1", tag=f"{nm}1")
                for mc in range(2):
                    for cc, (c0, csz, dst) in enumerate(
                            [(0, 128, d0), (128, 64, d1)]):
                        ps = p3p.tile([csz, 96], bf16, tag=f"trp{cc}")
                        T.transpose(ps[:, :], src_pair[mc][:, c0:c0 + csz],
                                    ident[0:96, 0:96])
                        S.copy(dst[:, mc * 96:(mc + 1) * 96], ps[:, :])
                return d0, d1

            wqsT = transpose_pair(wqs, "wqsT")
            wksT = transpose_pair(wks, "wksT")

            # AkT = X @ Wkn^T  (X symmetric)
            akT0 = p3s.tile([128, C], bf16, name="akT0", tag="akT0")
            akT1 = p3s.tile([64, C], bf16, name="akT1", tag="akT1")
            for (m0, msz, dst) in [(0, 128, akT0), (128, 64, akT1)]:
                ps = p3p.tile([msz, C], f32, tag="akTps")
                MM(ps[:, :], Xb[0][:, m0:m0 + msz], wksT[0][:, :], True, False)
                MM(ps[:, :], Xb[1][:, m0:m0 + msz], wksT[1][:, :], False, True)
                S.copy(dst[:, :], ps[:, :])

            # G = Wqn @ AkT ; masked full-row softmax (mask = -3e4 off own
            # head's 12x12 block -> exact zeros after exp) gives the
            # block-diagonal attention matrix rows directly.
            abdT = []
            for mc in range(2):
                msl = slice(mc * 96, (mc + 1) * 96)
                psG = p3p.tile([96, C], f32, tag="psG")
                MM(psG[:, :], wqsT[0][:, msl], akT0[:, :], True, False)
                MM(psG[:, :], wqsT[1][:, msl], akT1[:, :], False, True)
                gf = p3s.tile([96, C], f32, name=f"gf{mc}", tag=f"gf{mc}")
                V.tensor_tensor(gf[:, :], psG[:, :], bmask[mc][:, :], add)
                mx = p3s.tile([96, 1], f32, name=f"mx{mc}", tag=f"mx{mc}")
                V.tensor_reduce(mx[:, :], gf[:, :], AX.X, AluOpType.max)
                V.tensor_scalar_mul(mx[:, :], mx[:, :], -1.0)
                ex = p3s.tile([96, C], f32, name=f"ex{mc}", tag=f"ex{mc}")
                S.activation(ex[:, :], gf[:, :], AF.Exp, bias=mx[:, :])
                sm = p3s.tile([96, 1], f32, name=f"sm{mc}", tag=f"sm{mc}")
                V.tensor_reduce(sm[:, :], ex[:, :], AX.X, AluOpType.add)
                V.reciprocal(sm[:, :], sm[:, :])
                at = p3s.tile([96, C], bf16, name=f"at{mc}", tag=f"at{mc}")
                V.tensor_scalar_mul(at[:, :], ex[:, :], sm[:, :])
                pst = p3p.tile([96, 96], bf16, tag="attnT")
                T.transpose(pst[:, :], at[:, mc * 96:(mc + 1) * 96],
                            ident[0:96, 0:96])
                ab = vpool.tile([96, C], bf16, name=f"abdT{mc}", tag=f"abdT{mc}")
                V.memset(ab[:, :], 0)
                S.copy(ab[:, mc * 96:(mc + 1) * 96], pst[:, :])
                abdT.append(ab)

            # ---- W'vT = Wv^T @ A_bd^T  (tiny; folds attention into Wv) ----
            wvtT = (vpool.tile([128, C], bf16, name="wvtT0", tag="wvtT0"),
                    vpool.tile([64, C], bf16, name="wvtT1", tag="wvtT1"))
            for (m0, msz, dst) in [(0, 128, wvtT[0]), (128, 64, wvtT[1])]:
                ps = p3p.tile([msz, C], f32, tag="akTps")
                MM(ps[:, :], wvn[0][:, m0:m0 + msz], abdT[0][:, :], True, False)
                MM(ps[:, :], wvn[1][:, m0:m0 + msz], abdT[1][:, :], False, True)
                S.copy(dst[:, :], ps[:, :])

        # ================= phase 4+5: t, depthwise, proj =================
        ph5 = ExitStack()
        with ph5:
            pstr = ph5.enter_context(tc.tile_pool(name="stripes", bufs=1))
            tA = [pstr.tile([128, SFREE], bf16, name=f"tA{i}", tag=f"tA{i}")
                  for i in range(2)]
            tB = pstr.tile([128, SFREE], bf16, name="tB", tag="tB")
            ptps = ph5.enter_context(
                tc.tile_pool(name="tps", bufs=2, space="PSUM"))
            pyps = ph5.enter_context(
                tc.tile_pool(name="yps", bufs=2, space="PSUM"))
            pz = ph5.enter_context(tc.tile_pool(name="z", bufs=3))
            pzB = ph5.enter_context(tc.tile_pool(name="zB", bufs=NDT + 1))
            ptmp = ph5.enter_context(tc.tile_pool(name="dwtmp", bufs=4))
            pys = ph5.enter_context(tc.tile_pool(name="ystage", bufs=3))

            def st3(tile_):
                return tile_.rearrange("p (r c) -> p r c", c=TPW)

            # border zeros
            for st in tA:
                P.memset(st3(st)[:, 0:1, :], 0)         # top pad row
                P.memset(st3(st)[:, SROWS - 1:SROWS, :], 0)  # bottom pad row
            P.memset(st3(tB)[0:64, 0:1, :], 0)
            P.memset(st3(tB)[64:128, SROWS - 1:SROWS, :], 0)
            # the "real data" edge rows of each stripe get their pad row
            # overwritten by the dup-row evacs below except at the extremes;
            # memset all four stripe edge rows then col borders:
            P.memset(st3(tB)[0:64, SROWS - 1:SROWS, :], 0)
            P.memset(st3(tB)[64:128, 0:1, :], 0)
            for st in (tA[0], tA[1], tB):
                P.memset(st3(st)[:, :, 0:1], 0)
                P.memset(st3(st)[:, :, TPW - 1:TPW], 0)

            # ---- t = W'v @ x, evacuated into padded stripes ----
            def t_mm_B(ps, bsl2, sl):
                MM(ps[bsl2, :], wvtT[0][:, 128:C], xA[:, sl], True, False)
                MM(ps[bsl2, :], wvtT[1][:, 128:C], xB[:, sl], False, True)

            for j in range(NJT):
                half, jl = divmod(j, HJT)
                px = j * PT
                sl = slice(px, px + PT)
                stA = tA[half]
                psA = ptps.tile([128, PT], f32, tag="ptA")
                MM(psA[:, :], wvtT[0][:, 0:128], xA[:, sl], True, False)
                MM(psA[:, :], wvtT[1][:, 0:128], xB[:, sl], False, True)
                psB = ptps.tile([128, PT], f32, tag="ptB")
                bsl = slice(0, 64) if half == 0 else slice(64, 128)
                t_mm_B(psB, bsl, sl)

                r0 = jl * PR + 1  # local padded row of first row
                S.copy(st3(stA)[:, r0:r0 + PR, 1:1 + W_], psA[:, :])
                S.copy(st3(tB)[bsl, r0:r0 + PR, 1:1 + W_], psB[bsl, :])

                if j == HJT - 1:  # rows halfH-1: dup into bottom stripes row 0
                    S.copy(st3(tA[1])[:, 0:1, 1:1 + W_], psA[:, PT - W_:PT])
                    psBx = ptps.tile([128, PT], f32, tag="ptB")
                    t_mm_B(psBx, slice(64, 128), sl)
                    S.copy(st3(tB)[64:128, 0:1, 1:1 + W_], psBx[64:128, PT - W_:PT])
                if j == HJT:  # row halfH: dup into top stripes last row
                    S.copy(st3(tA[0])[:, SROWS - 1:SROWS, 1:1 + W_],
                           psA[:, 0:W_])
                    psBx = ptps.tile([128, PT], f32, tag="ptB")
                    t_mm_B(psBx, slice(0, 64), sl)
                    S.copy(st3(tB)[0:64, SROWS - 1:SROWS, 1:1 + W_],
                           psBx[0:64, 0:W_])

            # ---- depthwise + projection ----
            iACT = [_NON_PE.index(t_) for t_ in ACT_TAPS]
            iDVE = [_NON_PE.index(t_) for t_ in DVE_TAPS]

            def dw_tile(stripe, lanes, jd, dw, ztile):
                """emit the 7 non-PE taps for RT out-rows into ztile[lanes]"""
                s3 = st3(stripe)
                z3 = ztile.rearrange("p (r c) -> p r c", c=W_)

                def vw(k):
                    di, dj = _NON_PE[k]
                    return s3[lanes, jd * RT + di:jd * RT + di + RT, dj:dj + W_]

                zv = z3[lanes, :, :]
                k0 = iDVE[0]
                V.tensor_scalar_mul(zv, vw(k0), dw[lanes, k0:k0 + 1])
                for k in iDVE[1:]:
                    tm = ptmp.tile([128, RT * W_], bf16, tag="tmp")
                    t3 = tm.rearrange("p (r c) -> p r c", c=W_)[lanes, :, :]
                    V.tensor_scalar_mul(t3, vw(k), dw[lanes, k:k + 1])
                    V.tensor_tensor(zv, zv, t3, add)
                for k, eng in zip(iACT, ACT_MERGE):
                    tm = ptmp.tile([128, RT * W_], bf16, tag="tmp")
                    t3 = tm.rearrange("p (r c) -> p r c", c=W_)[lanes, :, :]
                    S.activation(t3, vw(k), AF.Copy, scale=dw[lanes, k:k + 1])
                    (P if eng == "pool" else V).tensor_tensor(zv, zv, t3, add)

            allL = slice(0, 128)
            for half in range(2):
                zB_tiles = {}
                for jt in range(HJT):
                    jd, part = divmod(jt, RPT)
                    j = half * HJT + jt
                    if part == 0:
                        zA = pz.tile([128, RT * W_], bf16, tag="zA")
                        dw_tile(tA[half], allL, jd, dwA, zA)
                        if half == 0:
                            zB = pzB.tile([128, RT * W_], bf16, tag="zB")
                            dw_tile(tB, allL, jd, dwB, zB)
                            zB_tiles[jd] = zB
                        else:
                            zB = zB_tiles_prev[jd]
                    zsl = slice(part * PT, (part + 1) * PT)
                    bsl = slice(0, 64) if half == 0 else slice(64, 128)
                    r0 = jt * PR  # local padded row base for PE tap views
                    ps_pair = []
                    for (m0, msz, pst) in [(0, 128, None), (128, 64, None)]:
                        psy = pyps.tile([msz, PT], f32, tag=f"py{m0}")
                        msl = slice(m0, m0 + msz)
                        MM(psy[:, :], wpT[0][:, msl], zA[:, zsl], True, False)
                        MM(psy[:, :], wpT[1][bsl, msl], zB[bsl, zsl],
                           False, False)
                        for i, (di, dj) in enumerate(PE_TAPS):
                            vA = st3(tA[half])[:, r0 + di:r0 + di + PR,
                                               dj:dj + W_]
                            vB = st3(tB)[bsl, r0 + di:r0 + di + PR, dj:dj + W_]
                            MM(psy[:, :], wps[i][0][:, msl], vA, False, False)
                            MM(psy[:, :], wps[i][1][bsl, msl], vB, False,
                               i == len(PE_TAPS) - 1)
                        ys = pys.tile([msz, PT], bf16, tag=f"ys{m0}")
                        S.copy(ys[:, :], psy[:, :])
                        dma(y_d[m0:m0 + msz, j * PT:(j + 1) * PT], ys[:, :])
                if half == 0:
                    zB_tiles_prev = zB_tiles
    return dram, y_d


# --------------------------------------------------------------------------
# host entry
# --------------------------------------------------------------------------

_CACHE = {}


def build_module(plan, num_devices=8):
    import concourse.bacc as bacc
    import concourse.tile as tile

    nc = bacc.Bacc("TRN2", target_bir_lowering=False, debug=False,
                   num_devices=num_devices)
    with tile.TileContext(nc) as tc:
        build_graph(nc, tc, plan)
    nc.compile()
    return nc


def _build_and_run(in_maps, plan):
    from concourse import bass_utils

    key = (plan["H"], plan["W"])
    if key not in _CACHE:
        _CACHE[key] = build_module(plan, num_devices=len(in_maps))
    nc = _CACHE[key]
    res = bass_utils.run_bass_kernel_spmd(
        nc, in_maps, core_ids=list(range(len(in_maps))))
    return res


def kernel(x, w_qkv, w_dw, w_proj, temperature):
    x = np.asarray(x, np.float32)
    plan = make_plan(H, W)
    prep = host_prep(w_qkv, w_dw, w_proj, temperature)
    xf = x.reshape(B, C, H * W)
    in_maps = [{"x": np.ascontiguousarray(xf[b]), **prep} for b in range(B)]
    res = _build_and_run(in_maps, plan)
    y = np.stack([np.asarray(r["y"], np.float32) for r in res.results])
    return y.reshape(B, C, H, W).astype(np.float32)
